# revision 16
# baseline (speedup 1.0000x reference)
"""Conv2d(128->256, 3x3, pad=1) over (32,128,56,56), data-parallel across 8
NeuronCores (4 images per core).

Per core: conv = 9 shifted accumulating matmuls per output tile.
  - contraction K = Cin = 128 (partition dim)
  - stationary lhsT = W^T[ci, co_tile] per (ky,kx)  -> [128, 128] bf16
  - moving rhs = input pixels [128, <=8 rows, <=56 cols] (N <= 448)
  - PSUM accumulates the 9 (ky,kx) taps; padding handled by clipping each
    tap's matmul to the valid rectangle (center tap goes first with
    start=True and covers the full tile, so partial-range taps accumulate
    on top via PSUM's per-element has_written bits).
Bias is added during the PSUM->SBUF copy (VectorE tensor_scalar), writing
bf16 (host converts the gathered output back to f32 -- the error is far
inside the bf16-matmul noise floor).

DMA layout (2 fast HWDGE rings: sync/scalar; gpsimd is a slow SWDGE
queue).  A ring round-robins among ALL dmas queued on it, so anything
sharing a ring with a critical-path load delays it:
  - sync carries ONLY the cot-0 weights first; scalar carries ONLY the
    first image in row quarters; gpsimd gets the bias (tiny) and image 3
    (needed last, fits the slow queue);
  - the remaining bulk loads (cot-1 weights, images 1-2) are issued
    between output stores inside the compute loop, so they enter the
    rings only after the critical pieces completed;
  - output stores alternate sync/scalar; the last block's final row
    chunk is split into two 4-row PSUM tiles so the first half's
    drain+store overlaps the second half's matmuls and the exit barrier
    waits only on a tiny final transfer on an otherwise-empty ring.
PE prewarm: dummy zero matmuls (memset on VectorE, which cannot issue
DMAs) bridge the entry barrier to the first data-dependent matmul and
warm the HAM clock-gate.
"""

import numpy as np
import ml_dtypes

import concourse.mybir as mybir
import concourse.tile as tile
from concourse import bacc
from concourse.bass_utils import run_bass_kernel_spmd

N_CORES = 8
B, CIN, H, W = 32, 128, 56, 56
COUT, R, S = 256, 3, 3
BL = B // N_CORES          # images per core
NCOT = COUT // 128         # Cout tiles of 128
YCHUNK = 8                 # output rows per matmul tile
NYC = H // YCHUNK

MM_DT = mybir.dt.bfloat16
MM_NP = ml_dtypes.bfloat16

NWARM = 7                  # dummy matmuls bridging entry barrier -> first data
X0_SPLITS = [(0, 16), (16, 36), (36, 56)]         # first-image load pieces (rows)
OUT_SPLITS = {1: (0, 14), 3: (14, 28), 5: (28, 48), 6: (48, 56)}  # yc -> store rows
# tap order in the weight layout: center tap first (it is the start=True
# matmul that covers the full PSUM tile)
TAP_ORDER = [(1, 1), (0, 0), (0, 1), (0, 2), (1, 0), (1, 2), (2, 0), (2, 1), (2, 2)]

_cache = {}


def _build():
    if "nc" in _cache:
        return _cache["nc"]
    nc = bacc.Bacc("TRN2", target_bir_lowering=False, debug=False)
    f32 = mybir.dt.float32
    x_d = nc.dram_tensor("x", [BL, CIN, H, W], MM_DT, kind="ExternalInput").ap()
    w_d = nc.dram_tensor("w", [CIN, NCOT, R * S, 128], MM_DT, kind="ExternalInput").ap()
    b_d = nc.dram_tensor("b", [128, NCOT], f32, kind="ExternalInput").ap()
    y_d = nc.dram_tensor("y", [BL, COUT, H, W], MM_DT, kind="ExternalOutput").ap()

    with tile.TileContext(nc) as tc:
        with (
            tc.tile_pool(name="consts", bufs=1) as cpool,
            tc.tile_pool(name="xin", bufs=BL) as xpool,
            tc.tile_pool(name="yout", bufs=2) as opool,
            tc.tile_pool(name="ps", bufs=8, space="PSUM") as pspool,
        ):
            # --- PE prewarm: zero matmuls with no DMA dependency ---
            warm_x = cpool.tile([128, 512], MM_DT)
            nc.vector.memset(warm_x[:], 0.0)
            warm_ps = pspool.tile([128, 512], f32, tag="ps")
            for _ in range(NWARM):
                nc.tensor.matmul(
                    warm_ps[:], warm_x[:, 0:128], warm_x[:], start=True, stop=True
                )

            # --- critical first loads, one per ring ---
            w_sb = cpool.tile([CIN, NCOT, R * S, 128], MM_DT)
            b_sb = cpool.tile([128, NCOT], f32)
            nc.sync.dma_start(w_sb[:, 0], w_d[:, 0])
            x_tiles = [xpool.tile([CIN, H, W], MM_DT, name="x_sb_0", tag="x_sb")]
            for r0, r1 in X0_SPLITS:
                nc.scalar.dma_start(x_tiles[0][:, r0:r1, :], x_d[0, :, r0:r1, :])
            nc.gpsimd.dma_start(b_sb[:], b_d[:])
            for img in range(1, BL):
                x_tiles.append(
                    xpool.tile([CIN, H, W], MM_DT, name=f"x_sb_{img}", tag="x_sb")
                )
            nc.gpsimd.dma_start(x_tiles[3][:], x_d[3])

            # bulk loads issued between stores inside the loop, so they hit
            # the rings only after the critical pieces are done
            deferred = [
                (nc.sync, w_sb[:, 1], w_d[:, 1]),
                (nc.scalar, x_tiles[1][:, 0:28, :], x_d[1, :, 0:28, :]),
                (nc.sync, x_tiles[1][:, 28:, :], x_d[1, :, 28:, :]),
                (nc.scalar, x_tiles[2][:, 0:28, :], x_d[2, :, 0:28, :]),
                (nc.sync, x_tiles[2][:, 28:, :], x_d[2, :, 28:, :]),
            ]

            store_rings = [nc.sync, nc.scalar]
            store_cnt = 0

            norm_chunks = [(YCHUNK * yc, YCHUNK) for yc in range(NYC)]
            norm_stores = dict(OUT_SPLITS)
            # last block: split the final row chunk in two so the first
            # half's drain+store overlaps the second half's matmuls and the
            # exit barrier waits only on a tiny final transfer
            last_chunks = norm_chunks[:-1] + [(48, 4), (52, 4)]
            last_stores = {1: (0, 14), 3: (14, 28), 5: (28, 48), 6: (48, 52), 7: (52, 56)}

            for img in range(BL):
                x_sb = x_tiles[img]
                for cot in range(NCOT):
                    last_block = img == BL - 1 and cot == NCOT - 1
                    chunks = last_chunks if last_block else norm_chunks
                    stores = last_stores if last_block else norm_stores
                    o_sb = opool.tile(
                        [128, H, W], MM_DT, name=f"o_sb_{img}_{cot}", tag="o_sb"
                    )
                    for yc, (y0, rows) in enumerate(chunks):
                        ps = pspool.tile(
                            [128, rows, W], f32, name=f"ps_{img}_{cot}_{yc}", tag="ps"
                        )
                        # center tap first: full-tile write with start=True
                        nc.tensor.matmul(
                            ps[:],
                            w_sb[:, cot, 0, :],
                            x_sb[:, y0 : y0 + rows, :],
                            start=True,
                            stop=False,
                        )
                        for ti, (ky, kx) in enumerate(TAP_ORDER[1:], start=1):
                            oy0 = max(0, 1 - ky - y0)
                            oy1 = min(rows, H + 1 - y0 - ky)
                            ox0 = max(0, 1 - kx)
                            ox1 = min(W, W + 1 - kx)
                            nc.tensor.matmul(
                                ps[:, oy0:oy1, ox0:ox1],
                                w_sb[:, cot, ti, :],
                                x_sb[
                                    :,
                                    y0 + oy0 + ky - 1 : y0 + oy1 + ky - 1,
                                    ox0 + kx - 1 : ox1 + kx - 1,
                                ],
                                start=False,
                                stop=(ti == R * S - 1),
                            )
                        # PSUM -> SBUF with fused bias add, all on VectorE
                        # (no ACTIVATE => Scalar never loads its LUT)
                        nc.vector.tensor_scalar_add(
                            o_sb[:, y0 : y0 + rows],
                            ps[:],
                            b_sb[:, cot : cot + 1],
                        )
                        # store finished row bands, alternating fast rings
                        if yc in stores:
                            r0, r1 = stores[yc]
                            eng = store_rings[store_cnt % 2]
                            store_cnt += 1
                            eng.dma_start(
                                y_d[img, 128 * cot : 128 * (cot + 1), r0:r1, :],
                                o_sb[:, r0:r1, :],
                            )
                            if deferred:
                                deng, dst, src = deferred.pop(0)
                                deng.dma_start(dst, src)

    nc.compile()
    _cache["nc"] = nc
    return nc


def _in_maps(inputs, weight, bias):
    x = np.asarray(inputs).astype(MM_NP)
    # weight (co, ci, ky, kx) -> (ci, cot, tap, co_in_tile), taps in TAP_ORDER
    wt = (
        np.asarray(weight)
        .reshape(NCOT, 128, CIN, R, S)
        .transpose(2, 0, 3, 4, 1)  # (ci, cot, ky, kx, co)
        .astype(MM_NP)
    )
    w = np.ascontiguousarray(
        np.stack([wt[:, :, ky, kx, :] for ky, kx in TAP_ORDER], axis=2)
    )
    b = np.ascontiguousarray(
        np.asarray(bias).astype(np.float32).reshape(NCOT, 128).T
    )
    return [
        {"x": np.ascontiguousarray(x[c * BL : (c + 1) * BL]), "w": w, "b": b}
        for c in range(N_CORES)
    ]


def kernel(inputs, weight, bias):
    nc = _build()
    in_maps = _in_maps(inputs, weight, bias)
    res = run_bass_kernel_spmd(nc, in_maps, core_ids=list(range(N_CORES)))
    out = np.concatenate([res.results[c]["y"] for c in range(N_CORES)], axis=0)
    return out.astype(np.float32)


# revision 17
# speedup vs baseline: 1.0279x; 1.0279x over previous
"""Conv2d(128->256, 3x3, pad=1) over (32,128,56,56), data-parallel across 8
NeuronCores (4 images per core).

Per core: conv = 9 shifted accumulating matmuls per output tile.
  - contraction K = Cin = 128 (partition dim)
  - stationary lhsT = W^T[ci, co_tile] per (ky,kx)  -> [128, 128] bf16
  - moving rhs = input pixels [128, <=8 rows, <=56 cols] (N <= 448)
  - PSUM accumulates the 9 (ky,kx) taps; padding handled by clipping each
    tap's matmul to the valid rectangle (center tap goes first with
    start=True and covers the full tile, so partial-range taps accumulate
    on top via PSUM's per-element has_written bits).
Bias is added during the PSUM->SBUF copy (VectorE tensor_scalar), writing
bf16 (host converts the gathered output back to f32 -- the error is far
inside the bf16-matmul noise floor).

DMA layout (2 fast HWDGE rings: sync/scalar; gpsimd is a slow SWDGE
queue).  A ring round-robins among ALL dmas queued on it, so anything
sharing a ring with a critical-path load delays it:
  - sync carries ONLY the cot-0 weights first; scalar carries ONLY the
    first image in row quarters; gpsimd gets the bias (tiny) and image 3
    (needed last, fits the slow queue);
  - the remaining bulk loads (cot-1 weights, images 1-2) are issued
    between output stores inside the compute loop, so they enter the
    rings only after the critical pieces completed;
  - output stores alternate sync/scalar; the last block's final row
    chunk is split into two 4-row PSUM tiles so the first half's
    drain+store overlaps the second half's matmuls and the exit barrier
    waits only on a tiny final transfer on an otherwise-empty ring.
PE prewarm: dummy zero matmuls (memset on VectorE, which cannot issue
DMAs) bridge the entry barrier to the first data-dependent matmul and
warm the HAM clock-gate.
"""

import numpy as np
import ml_dtypes

import concourse.mybir as mybir
import concourse.tile as tile
from concourse import bacc
from concourse.bass_utils import run_bass_kernel_spmd

N_CORES = 8
B, CIN, H, W = 32, 128, 56, 56
COUT, R, S = 256, 3, 3
BL = B // N_CORES          # images per core
NCOT = COUT // 128         # Cout tiles of 128
YCHUNK = 8                 # output rows per matmul tile
NYC = H // YCHUNK

MM_DT = mybir.dt.bfloat16
MM_NP = ml_dtypes.bfloat16

NWARM = 8                  # dummy matmuls bridging entry barrier -> first data
# piece 1 covers the first three row chunks, so the stream's cadence only
# depends on piece-2 timing ~5us in (robust to DMA-latency jitter)
X0_SPLITS = [(0, 26), (26, 42), (42, 56)]         # first-image load pieces (rows)
OUT_SPLITS = {1: (0, 14), 3: (14, 28), 5: (28, 48), 6: (48, 56)}  # yc -> store rows
# tap order in the weight layout: center tap first (it is the start=True
# matmul that covers the full PSUM tile)
TAP_ORDER = [(1, 1), (0, 0), (0, 1), (0, 2), (1, 0), (1, 2), (2, 0), (2, 1), (2, 2)]

_cache = {}


def _build():
    if "nc" in _cache:
        return _cache["nc"]
    nc = bacc.Bacc("TRN2", target_bir_lowering=False, debug=False)
    f32 = mybir.dt.float32
    x_d = nc.dram_tensor("x", [BL, CIN, H, W], MM_DT, kind="ExternalInput").ap()
    w_d = nc.dram_tensor("w", [CIN, NCOT, R * S, 128], MM_DT, kind="ExternalInput").ap()
    b_d = nc.dram_tensor("b", [128, NCOT], f32, kind="ExternalInput").ap()
    y_d = nc.dram_tensor("y", [BL, COUT, H, W], MM_DT, kind="ExternalOutput").ap()

    with tile.TileContext(nc) as tc:
        with (
            tc.tile_pool(name="consts", bufs=1) as cpool,
            tc.tile_pool(name="xin", bufs=BL) as xpool,
            tc.tile_pool(name="yout", bufs=2) as opool,
            tc.tile_pool(name="ps", bufs=8, space="PSUM") as pspool,
        ):
            # --- PE prewarm: zero matmuls with no DMA dependency ---
            warm_x = cpool.tile([128, 512], MM_DT)
            nc.vector.memset(warm_x[:], 0.0)
            warm_ps = pspool.tile([128, 512], f32, tag="ps")
            for _ in range(NWARM):
                nc.tensor.matmul(
                    warm_ps[:], warm_x[:, 0:128], warm_x[:], start=True, stop=True
                )

            # --- critical first loads, one per ring ---
            w_sb = cpool.tile([CIN, NCOT, R * S, 128], MM_DT)
            b_sb = cpool.tile([128, NCOT], f32)
            nc.sync.dma_start(w_sb[:, 0], w_d[:, 0])
            x_tiles = [xpool.tile([CIN, H, W], MM_DT, name="x_sb_0", tag="x_sb")]
            for r0, r1 in X0_SPLITS:
                nc.scalar.dma_start(x_tiles[0][:, r0:r1, :], x_d[0, :, r0:r1, :])
            nc.gpsimd.dma_start(b_sb[:], b_d[:])
            for img in range(1, BL):
                x_tiles.append(
                    xpool.tile([CIN, H, W], MM_DT, name=f"x_sb_{img}", tag="x_sb")
                )
            nc.gpsimd.dma_start(x_tiles[3][:], x_d[3])

            # bulk loads issued between stores inside the loop, so they hit
            # the rings only after the critical pieces are done
            deferred = [
                (nc.sync, w_sb[:, 1], w_d[:, 1]),
                (nc.scalar, x_tiles[1][:, 0:28, :], x_d[1, :, 0:28, :]),
                (nc.sync, x_tiles[1][:, 28:, :], x_d[1, :, 28:, :]),
                (nc.scalar, x_tiles[2][:, 0:28, :], x_d[2, :, 0:28, :]),
                (nc.sync, x_tiles[2][:, 28:, :], x_d[2, :, 28:, :]),
            ]

            store_rings = [nc.sync, nc.scalar]
            store_cnt = 0

            norm_chunks = [(YCHUNK * yc, YCHUNK) for yc in range(NYC)]
            norm_stores = dict(OUT_SPLITS)
            # last block: split the final row chunk in two so the first
            # half's drain+store overlaps the second half's matmuls and the
            # exit barrier waits only on a tiny final transfer
            last_chunks = norm_chunks[:-1] + [(48, 4), (52, 4)]
            last_stores = {1: (0, 14), 3: (14, 28), 5: (28, 48), 6: (48, 52), 7: (52, 56)}

            for img in range(BL):
                x_sb = x_tiles[img]
                for cot in range(NCOT):
                    last_block = img == BL - 1 and cot == NCOT - 1
                    chunks = last_chunks if last_block else norm_chunks
                    stores = last_stores if last_block else norm_stores
                    o_sb = opool.tile(
                        [128, H, W], MM_DT, name=f"o_sb_{img}_{cot}", tag="o_sb"
                    )
                    for yc, (y0, rows) in enumerate(chunks):
                        ps = pspool.tile(
                            [128, rows, W], f32, name=f"ps_{img}_{cot}_{yc}", tag="ps"
                        )
                        # center tap first: full-tile write with start=True
                        nc.tensor.matmul(
                            ps[:],
                            w_sb[:, cot, 0, :],
                            x_sb[:, y0 : y0 + rows, :],
                            start=True,
                            stop=False,
                        )
                        for ti, (ky, kx) in enumerate(TAP_ORDER[1:], start=1):
                            oy0 = max(0, 1 - ky - y0)
                            oy1 = min(rows, H + 1 - y0 - ky)
                            ox0 = max(0, 1 - kx)
                            ox1 = min(W, W + 1 - kx)
                            nc.tensor.matmul(
                                ps[:, oy0:oy1, ox0:ox1],
                                w_sb[:, cot, ti, :],
                                x_sb[
                                    :,
                                    y0 + oy0 + ky - 1 : y0 + oy1 + ky - 1,
                                    ox0 + kx - 1 : ox1 + kx - 1,
                                ],
                                start=False,
                                stop=(ti == R * S - 1),
                            )
                        # PSUM -> SBUF with fused bias add, all on VectorE
                        # (no ACTIVATE => Scalar never loads its LUT)
                        nc.vector.tensor_scalar_add(
                            o_sb[:, y0 : y0 + rows],
                            ps[:],
                            b_sb[:, cot : cot + 1],
                        )
                        # store finished row bands, alternating fast rings
                        if yc in stores:
                            r0, r1 = stores[yc]
                            eng = store_rings[store_cnt % 2]
                            store_cnt += 1
                            eng.dma_start(
                                y_d[img, 128 * cot : 128 * (cot + 1), r0:r1, :],
                                o_sb[:, r0:r1, :],
                            )
                            if deferred:
                                deng, dst, src = deferred.pop(0)
                                deng.dma_start(dst, src)

    nc.compile()
    _cache["nc"] = nc
    return nc


def _in_maps(inputs, weight, bias):
    x = np.asarray(inputs).astype(MM_NP)
    # weight (co, ci, ky, kx) -> (ci, cot, tap, co_in_tile), taps in TAP_ORDER
    wt = (
        np.asarray(weight)
        .reshape(NCOT, 128, CIN, R, S)
        .transpose(2, 0, 3, 4, 1)  # (ci, cot, ky, kx, co)
        .astype(MM_NP)
    )
    w = np.ascontiguousarray(
        np.stack([wt[:, :, ky, kx, :] for ky, kx in TAP_ORDER], axis=2)
    )
    b = np.ascontiguousarray(
        np.asarray(bias).astype(np.float32).reshape(NCOT, 128).T
    )
    return [
        {"x": np.ascontiguousarray(x[c * BL : (c + 1) * BL]), "w": w, "b": b}
        for c in range(N_CORES)
    ]


def kernel(inputs, weight, bias):
    nc = _build()
    in_maps = _in_maps(inputs, weight, bias)
    res = run_bass_kernel_spmd(nc, in_maps, core_ids=list(range(N_CORES)))
    out = np.concatenate([res.results[c]["y"] for c in range(N_CORES)], axis=0)
    return out.astype(np.float32)
